# revision 16
# baseline (speedup 1.0000x reference)
"""Multi-head attention layer (B=4, S=2048, D=1024, H=16, DH=64) on 8 TRN2 cores.

Sharding: core c = (batch b, head-group g) with b = c//2, g = c%2.
Each core computes QKV projections for one batch with an 8-head column slice
of the weights, then full attention for those 8 heads — zero collectives.

Per-core layout choices:
  - x is passed host-transposed as xT (D, S) so the D-contraction sits on
    SBUF partitions for all three projection matmuls without device transposes.
  - key/value sequence compaction: masked key positions contribute exactly 0
    to the softmax (exp(-10000 + s) == 0 in f32), so the host gathers the
    unmasked positions (padded with masked ones to a static NKV) into a
    separate compacted operand xkT used for the K/V projections. Scores, exp
    and context shrink by NKV/S. NKV=1280 covers any mask with <=1280 active
    keys (a +11 sigma event for the Bernoulli(0.5) mask); kernel() falls back
    to an NKV=2048 graph otherwise, which is correct for any mask.
  - q, k are produced transposed (qT/kT: head_dim on partitions, seq free) so
    the scores matmul contracts over DH=64 directly; two heads share the PE
    array via row tile_position packing (base partitions 0 / 64).
  - scores are computed transposed (k on partitions, q free); the mask adder
    is a per-partition bias and the softmax exp is a single ScalarE
    activation (exp(0.125*s + adder)) straight out of PSUM.
  - v is produced in natural layout (kv position on partitions) with a
    ones-column appended per head, so the context matmul (M=65) yields the
    softmax denominators in PSUM row 64 for free.
  - normalization: the denominator row is broadcast across partitions with a
    K=1 outer-product matmul, reciprocal'd on DVE, and applied during the
    PSUM evict multiply. Output is written as ctxT (HD, S); host transposes.

All matmuls run as float32r (full-rate fp32 storage on the PE array).

`reps` repeats the whole compute body inside one NEFF (used only for
device-time measurement: wall(reps=k) - wall(reps=1) isolates body time
from host/RPC overhead).
"""

import os
import sys

import numpy as np

sys.path.insert(0, "/opt/trn_rl_repo")

B, S, D = 4, 2048, 1024
H, DH = 16, 64
HPC = 8            # heads per core
HD = HPC * DH      # 512: output columns per core
NCORES = 8
KD = D // 128      # 8 contraction chunks
NT = HD // 128     # 4 head-dim partition chunks (= head pairs)
NC4 = S // 512     # 4 q chunks of 512
VW = DH + 1        # 65: v columns per head incl. ones column
NKV_LADDER = (1280, 2048)  # compacted kv lengths

_CACHED = {}


def _pieces(n):
    """Split n into pieces of 512 with any tail split so every piece is a
    multiple of 128 and >= 256 (f32r matmuls need a moving dim >= 256 for
    full rate)."""
    out, off = [], 0
    while n - off > 768:
        out.append((off, 512))
        off += 512
    rem = n - off
    if rem > 512:
        out.append((off, rem - 256))
        out.append((off + rem - 256, 256))
    elif rem:
        out.append((off, rem))
    assert all(w >= 256 and w % 128 == 0 for _, w in out), out
    return out


def _build_nc(nkv, taps=False, reps=1):
    from concourse import bacc, mybir, tile

    f32 = mybir.dt.float32
    f32r = mybir.dt.float32r
    i32 = mybir.dt.int32
    EXP = mybir.ActivationFunctionType.Exp
    MULT = mybir.AluOpType.mult
    ADD = mybir.AluOpType.add

    NMK = nkv // 128          # kv chunks of 128
    kv_pieces = _pieces(nkv)

    nc = bacc.Bacc("TRN2", target_bir_lowering=False, debug=False,
                   enable_asserts=False)

    xt_d = nc.declare_dram_parameter("xt", [D, S], f32r, isOutput=False)
    xkt_d = nc.declare_dram_parameter("xkt", [D, nkv], f32r, isOutput=False)
    wq_d = nc.declare_dram_parameter("wq", [D, HD], f32r, isOutput=False)
    wk_d = nc.declare_dram_parameter("wk", [D, HD], f32r, isOutput=False)
    wv_d = nc.declare_dram_parameter("wv", [D, HD], f32r, isOutput=False)
    bq_d = nc.declare_dram_parameter("bq", [HD], f32, isOutput=False)
    bk_d = nc.declare_dram_parameter("bk", [HD], f32, isOutput=False)
    bv_d = nc.declare_dram_parameter("bv", [HD], f32r, isOutput=False)
    mask_d = nc.declare_dram_parameter("maskc", [nkv], i32, isOutput=False)
    out_d = nc.declare_dram_parameter("out", [HD, S], f32, isOutput=True)
    if taps:
        dqt_d = nc.declare_dram_parameter("dqt", [128, NT * S], f32r,
                                          isOutput=True)
        dkt_d = nc.declare_dram_parameter("dkt", [128, NT * nkv], f32r,
                                          isOutput=True)
        dva_d = nc.declare_dram_parameter("dva", [128, NMK * HPC * VW], f32r,
                                          isOutput=True)

    with tile.TileContext(nc) as tc:
        with (
            tc.tile_pool(name="const", bufs=1) as cpool,
            tc.tile_pool(name="qk", bufs=1) as qkpool,
            tc.tile_pool(name="vv", bufs=1) as vpool,
            tc.tile_pool(name="outp", bufs=4) as opool,
            tc.tile_pool(name="bcp", bufs=2) as bcpool,
            tc.tile_pool(name="rcp", bufs=2) as rcpool,
        ):
            ones_f = cpool.tile([128, 128], f32)
            nc.vector.memset(ones_f[:], 1.0)
            ones_r = cpool.tile([1, 128], f32r)
            nc.vector.tensor_copy(ones_r[:], ones_f[0:1, :])
            ones65_r = cpool.tile([VW, 128], f32r)
            nc.vector.tensor_copy(ones65_r[DH:DH + 1, :],
                                  ones_f[DH:DH + 1, :])

            # compacted mask (nkv,) int32 -> additive bias tile (128, NMK):
            # adder[p, m] = (maskc[m*128+p] - 1) * 10000
            mask_t = cpool.tile([128, NMK], i32)
            nc.sync.dma_start(
                mask_t[:], mask_d.ap().rearrange("(m p) -> p m", p=128))
            maskf = cpool.tile([128, NMK], f32)
            nc.vector.tensor_copy(maskf[:], mask_t[:])
            adder = cpool.tile([128, NMK], f32)
            nc.vector.tensor_scalar(adder[:], maskf[:], 10000.0, -10000.0,
                                    MULT, ADD)

            # biases: bq/bk as per-partition columns, bv as a 1-row vector
            bq_t = cpool.tile([128, NT], f32)
            nc.sync.dma_start(
                bq_t[:], bq_d.ap().rearrange("(t p) -> p t", p=128))
            bk_t = cpool.tile([128, NT], f32)
            nc.sync.dma_start(
                bk_t[:], bk_d.ap().rearrange("(t p) -> p t", p=128))
            bv_r = cpool.tile([1, HD], f32r)
            nc.sync.dma_start(bv_r[:],
                              bv_d.ap().rearrange("(o n) -> o n", o=1))

            qT = qkpool.tile([128, NT * S], f32r)    # q transposed
            kT = qkpool.tile([128, NT * nkv], f32r)  # k transposed, compacted
            vA = vpool.tile([128, NMK * HPC * VW], f32r)  # v' with ones cols
            nc.vector.tensor_copy(
                vA[:].rearrange("p (m h e) -> p m h e", m=NMK, h=HPC)
                [:, :, :, DH:DH + 1],
                ones_f[:, 0:NMK * HPC].rearrange(
                    "p (m h e) -> p m h e", m=NMK, h=HPC))

            xt_bufs = 16 if nkv < S else 8
            xk_bufs = 12 if nkv < S else 8

            for rep in range(reps):
                # ---------------- QKV projection phase ----------------
                with (
                    tc.tile_pool(name=f"xtp{rep}", bufs=xt_bufs) as xpool,
                    tc.tile_pool(name=f"xkp{rep}", bufs=xk_bufs) as xkpool,
                    tc.tile_pool(name=f"wp{rep}", bufs=1) as wpool,
                    tc.tile_pool(name=f"psq{rep}", bufs=4,
                                 space="PSUM") as psq,
                ):
                    wqt = wpool.tile([128, KD * HD], f32r)
                    wkt = wpool.tile([128, KD * HD], f32r)
                    wvt = wpool.tile([128, KD * HD], f32r)
                    for wt, wd in ((wqt, wq_d), (wkt, wk_d), (wvt, wv_d)):
                        for d in range(KD):
                            nc.sync.dma_start(
                                wt[:, d * HD:(d + 1) * HD],
                                wd.ap()[d * 128:(d + 1) * 128, :])

                    # k, v: compacted sequence
                    for off, w in kv_pieces:
                        xkp = []
                        for d in range(KD):
                            t_ = xkpool.tile([128, 512], f32r, tag="xk")
                            nc.sync.dma_start(
                                t_[:, 0:w],
                                xkt_d.ap()[d * 128:(d + 1) * 128,
                                           off:off + w])
                            xkp.append(t_)
                        for t in range(NT):
                            ps = psq.tile([128, 512], f32, tag="psqkv")
                            for d in range(KD):
                                nc.tensor.matmul(
                                    ps[:, 0:w],
                                    wkt[:, d * HD + t * 128:
                                        d * HD + (t + 1) * 128],
                                    xkp[d][:, 0:w],
                                    start=(d == 0), stop=(d == KD - 1))
                            nc.vector.tensor_scalar_add(
                                kT[:, t * nkv + off:t * nkv + off + w],
                                ps[:, 0:w], bk_t[:, t:t + 1])
                        for mi in range(w // 128):
                            m = off // 128 + mi
                            ps = psq.tile([128, 512], f32, tag="psqkv")
                            for d in range(KD):
                                nc.tensor.matmul(
                                    ps[:],
                                    xkp[d][:, mi * 128:(mi + 1) * 128],
                                    wvt[:, d * HD:(d + 1) * HD],
                                    start=(d == 0), stop=False)
                            nc.tensor.matmul(
                                ps[:], ones_r[:], bv_r[:],
                                start=False, stop=True)
                            nc.vector.tensor_copy(
                                vA[:, m * HPC * VW:(m + 1) * HPC * VW]
                                .rearrange("p (h e) -> p h e",
                                           h=HPC)[:, :, 0:DH],
                                ps[:].rearrange("p (h e) -> p h e", h=HPC))

                    # q: full sequence
                    for c4 in range(NC4):
                        xp = []
                        for d in range(KD):
                            t_ = xpool.tile([128, 512], f32r, tag="xt")
                            nc.sync.dma_start(
                                t_[:],
                                xt_d.ap()[d * 128:(d + 1) * 128,
                                          c4 * 512:(c4 + 1) * 512])
                            xp.append(t_)
                        for t in range(NT):
                            ps = psq.tile([128, 512], f32, tag="psqkv")
                            for d in range(KD):
                                nc.tensor.matmul(
                                    ps[:],
                                    wqt[:, d * HD + t * 128:
                                        d * HD + (t + 1) * 128],
                                    xp[d][:],
                                    start=(d == 0), stop=(d == KD - 1))
                            nc.vector.tensor_scalar_add(
                                qT[:, t * S + c4 * 512:
                                   t * S + (c4 + 1) * 512],
                                ps[:], bq_t[:, t:t + 1])

                if taps:
                    nc.sync.dma_start(dqt_d.ap(), qT[:])
                    nc.sync.dma_start(dkt_d.ap(), kT[:])
                    nc.sync.dma_start(dva_d.ap(), vA[:])

                # ---------------- attention phase ----------------
                with (
                    tc.tile_pool(name=f"probs{rep}", bufs=6) as ppool,
                    tc.tile_pool(name=f"pssc{rep}", bufs=2,
                                 space="PSUM") as pssc,
                    tc.tile_pool(name=f"psctx{rep}", bufs=3,
                                 space="PSUM") as psctx,
                    tc.tile_pool(name=f"psbc{rep}", bufs=1,
                                 space="PSUM") as psbc,
                ):
                    for g in range(NT):        # head pair (qT/kT chunk)
                        for c in range(NC4):   # q chunk of 512
                            ctxA = psctx.tile([VW, 512], f32, tag="ctx")
                            ctxB = psctx.tile([VW, 512], f32, tag="ctx")
                            for m in range(NMK):
                                sc = pssc.tile([128, 1024], f32, tag="sc")
                                # scoresT for the pair, row-packed on the PE
                                nc.tensor.matmul(
                                    sc[:, 0:512],
                                    kT[0:64, g * nkv + m * 128:
                                       g * nkv + (m + 1) * 128],
                                    qT[0:64, g * S + c * 512:
                                       g * S + (c + 1) * 512],
                                    start=True, stop=True)
                                nc.tensor.matmul(
                                    sc[:, 512:1024],
                                    kT[64:128, g * nkv + m * 128:
                                       g * nkv + (m + 1) * 128],
                                    qT[64:128, g * S + c * 512:
                                       g * S + (c + 1) * 512],
                                    start=True, stop=True)
                                probs = ppool.tile([128, 1024], f32r,
                                                   tag="probs")
                                nc.scalar.activation(
                                    probs[:], sc[:], EXP,
                                    bias=adder[:, m:m + 1], scale=0.125)
                                hA, hB = 2 * g, 2 * g + 1
                                nc.tensor.matmul(
                                    ctxA[:],
                                    vA[:, m * HPC * VW + hA * VW:
                                       m * HPC * VW + (hA + 1) * VW],
                                    probs[:, 0:512],
                                    start=(m == 0), stop=(m == NMK - 1))
                                nc.tensor.matmul(
                                    ctxB[:],
                                    vA[:, m * HPC * VW + hB * VW:
                                       m * HPC * VW + (hB + 1) * VW],
                                    probs[:, 512:1024],
                                    start=(m == 0), stop=(m == NMK - 1))

                            for h, ctx in ((2 * g, ctxA), (2 * g + 1, ctxB)):
                                # raw sums row (partition 64) -> SBUF, then
                                # broadcast to partitions 0..63 via a K=1
                                # matmul (the PE handles base-64 operands;
                                # the custom DVE recip reads partition 0),
                                # then reciprocal on the base-0 tile.
                                sums_r = rcpool.tile([VW, 512], f32r,
                                                     tag="sums")
                                nc.vector.tensor_copy(
                                    sums_r[DH:DH + 1, :], ctx[DH:DH + 1, :])
                                bc = psbc.tile([DH, 512], f32, tag="bc")
                                nc.tensor.matmul(
                                    bc[:], ones65_r[DH:DH + 1, 0:DH],
                                    sums_r[DH:DH + 1, :],
                                    start=True, stop=True)
                                bcs = bcpool.tile([DH, 512], f32, tag="bcs")
                                nc.vector.reciprocal_approx_fast(
                                    out=bcs[:], in_=bc[:])
                                o = opool.tile([DH, 512], f32, tag="o")
                                nc.vector.tensor_mul(o[:], ctx[0:DH, :],
                                                     bcs[:])
                                nc.sync.dma_start(
                                    out_d.ap()[h * DH:(h + 1) * DH,
                                               c * 512:(c + 1) * 512], o[:])

    nc.compile()
    return nc


def get_nc(nkv=NKV_LADDER[0]):
    if nkv not in _CACHED:
        _CACHED[nkv] = _build_nc(nkv)
    return _CACHED[nkv]


def make_in_maps(nkv, x, mask, wq, bq, wk, bk, wv, bv):
    x = np.ascontiguousarray(np.asarray(x, dtype=np.float32))
    mask = np.ascontiguousarray(np.asarray(mask, dtype=np.int32))
    wq = np.asarray(wq, dtype=np.float32)
    wk = np.asarray(wk, dtype=np.float32)
    wv = np.asarray(wv, dtype=np.float32)
    bq = np.asarray(bq, dtype=np.float32)
    bk = np.asarray(bk, dtype=np.float32)
    bv = np.asarray(bv, dtype=np.float32)
    # per-batch kv compaction indices (unmasked first, masked as padding)
    idx = []
    for b in range(B):
        on = np.flatnonzero(mask[b] != 0)
        off = np.flatnonzero(mask[b] == 0)
        ib = np.concatenate([on, off])[:nkv]
        idx.append(ib)
    in_maps = []
    for c in range(NCORES):
        b, g = c // 2, c % 2
        cols = slice(g * HD, (g + 1) * HD)
        xtb = np.ascontiguousarray(x[b].T)
        in_maps.append({
            "xt": xtb,
            "xkt": np.ascontiguousarray(xtb[:, idx[b]]),
            "wq": np.ascontiguousarray(wq[:, cols]),
            "wk": np.ascontiguousarray(wk[:, cols]),
            "wv": np.ascontiguousarray(wv[:, cols]),
            "bq": np.ascontiguousarray(bq[cols]),
            "bk": np.ascontiguousarray(bk[cols]),
            "bv": np.ascontiguousarray(bv[cols]),
            "maskc": np.ascontiguousarray(mask[b][idx[b]]),
        })
    return in_maps


def assemble_out(results):
    out = np.empty((B, S, H * DH), dtype=np.float32)
    for c in range(NCORES):
        b, g = c // 2, c % 2
        out[b, :, g * HD:(g + 1) * HD] = results[c]["out"].T
    return out


def pick_nkv(mask):
    mask = np.asarray(mask)
    nb_max = int((mask != 0).sum(axis=1).max())
    for nkv in NKV_LADDER:
        if nb_max <= nkv:
            return nkv
    return S


def run(trace=False, **inputs):
    from concourse.bass_utils import run_bass_kernel_spmd

    nkv = pick_nkv(inputs["mask"])
    nc = get_nc(nkv)
    in_maps = make_in_maps(nkv, **inputs)
    res = run_bass_kernel_spmd(nc, in_maps, core_ids=list(range(NCORES)),
                               trace=trace)
    return assemble_out(res.results), res


def kernel(**inputs):
    out, _ = run(trace=False, **inputs)
    return out


# revision 17
# speedup vs baseline: 1.0035x; 1.0035x over previous
"""Multi-head attention layer (B=4, S=2048, D=1024, H=16, DH=64) on 8 TRN2 cores.

Sharding: core c = (batch b, head-group g) with b = c//2, g = c%2.
Each core computes QKV projections for one batch with an 8-head column slice
of the weights, then full attention for those 8 heads — zero collectives.

Per-core layout choices:
  - x is passed host-transposed as xT (D, S) so the D-contraction sits on
    SBUF partitions for all three projection matmuls without device transposes.
  - key/value sequence compaction: masked key positions contribute exactly 0
    to the softmax (exp(-10000 + s) == 0 in f32), so the host gathers the
    unmasked positions (padded with masked ones to a static NKV) into a
    separate compacted operand xkT used for the K/V projections. Scores, exp
    and context shrink by NKV/S. NKV=1280 covers any mask with <=1280 active
    keys (a +11 sigma event for the Bernoulli(0.5) mask); kernel() falls back
    to an NKV=2048 graph otherwise, which is correct for any mask.
  - q, k are produced transposed (qT/kT: head_dim on partitions, seq free) so
    the scores matmul contracts over DH=64 directly; two heads share the PE
    array via row tile_position packing (base partitions 0 / 64).
  - scores are computed transposed (k on partitions, q free); the mask adder
    is a per-partition bias and the softmax exp is a single ScalarE
    activation (exp(0.125*s + adder)) straight out of PSUM.
  - v is produced in natural layout (kv position on partitions) with a
    ones-column appended per head, so the context matmul (M=65) yields the
    softmax denominators in PSUM row 64 for free.
  - normalization: the denominator row is broadcast across partitions with a
    K=1 outer-product matmul, reciprocal'd on DVE, and applied during the
    PSUM evict multiply. Output is written as ctxT (HD, S); host transposes.

All matmuls run as float32r (full-rate fp32 storage on the PE array).

`reps` repeats the whole compute body inside one NEFF (used only for
device-time measurement: wall(reps=k) - wall(reps=1) isolates body time
from host/RPC overhead).
"""

import os
import sys

import numpy as np

sys.path.insert(0, "/opt/trn_rl_repo")

B, S, D = 4, 2048, 1024
H, DH = 16, 64
HPC = 8            # heads per core
HD = HPC * DH      # 512: output columns per core
NCORES = 8
KD = D // 128      # 8 contraction chunks
NT = HD // 128     # 4 head-dim partition chunks (= head pairs)
NC4 = S // 512     # 4 q chunks of 512
VW = DH + 1        # 65: v columns per head incl. ones column

_CACHED = {}


def _pieces(n):
    """Split n into pieces of 512 with any tail split so every piece is a
    multiple of 128 and >= 256 (f32r matmuls need a moving dim >= 256 for
    full rate)."""
    out, off = [], 0
    while n - off > 768:
        out.append((off, 512))
        off += 512
    rem = n - off
    if rem > 512:
        out.append((off, rem - 256))
        out.append((off + rem - 256, 256))
    elif rem:
        out.append((off, rem))
    assert all(w >= 256 and w % 128 == 0 for _, w in out), out
    return out


def _build_nc(nkv, nmk_attn=None, taps=False, reps=1):
    from concourse import bacc, mybir, tile

    f32 = mybir.dt.float32
    f32r = mybir.dt.float32r
    i32 = mybir.dt.int32
    EXP = mybir.ActivationFunctionType.Exp
    MULT = mybir.AluOpType.mult
    ADD = mybir.AluOpType.add

    NMK = nkv // 128          # kv chunks of 128 (projection width)
    if nmk_attn is None:
        nmk_attn = NMK        # attention window in 128-chunks
    assert nmk_attn <= NMK
    kv_pieces = _pieces(nkv)

    nc = bacc.Bacc("TRN2", target_bir_lowering=False, debug=False,
                   enable_asserts=False)

    xt_d = nc.declare_dram_parameter("xt", [D, S], f32r, isOutput=False)
    xkt_d = nc.declare_dram_parameter("xkt", [D, nkv], f32r, isOutput=False)
    wq_d = nc.declare_dram_parameter("wq", [D, HD], f32r, isOutput=False)
    wk_d = nc.declare_dram_parameter("wk", [D, HD], f32r, isOutput=False)
    wv_d = nc.declare_dram_parameter("wv", [D, HD], f32r, isOutput=False)
    bq_d = nc.declare_dram_parameter("bq", [HD], f32, isOutput=False)
    bk_d = nc.declare_dram_parameter("bk", [HD], f32, isOutput=False)
    bv_d = nc.declare_dram_parameter("bv", [HD], f32r, isOutput=False)
    mask_d = nc.declare_dram_parameter("maskc", [nkv], i32, isOutput=False)
    out_d = nc.declare_dram_parameter("out", [HD, S], f32, isOutput=True)
    if taps:
        dqt_d = nc.declare_dram_parameter("dqt", [128, NT * S], f32r,
                                          isOutput=True)
        dkt_d = nc.declare_dram_parameter("dkt", [128, NT * nkv], f32r,
                                          isOutput=True)
        dva_d = nc.declare_dram_parameter("dva", [128, NMK * HPC * VW], f32r,
                                          isOutput=True)

    with tile.TileContext(nc) as tc:
        with (
            tc.tile_pool(name="const", bufs=1) as cpool,
            tc.tile_pool(name="qk", bufs=1) as qkpool,
            tc.tile_pool(name="vv", bufs=1) as vpool,
            tc.tile_pool(name="outp", bufs=4) as opool,
            tc.tile_pool(name="bcp", bufs=2) as bcpool,
            tc.tile_pool(name="rcp", bufs=2) as rcpool,
        ):
            ones_f = cpool.tile([128, 128], f32)
            nc.vector.memset(ones_f[:], 1.0)
            ones_r = cpool.tile([1, 128], f32r)
            nc.vector.tensor_copy(ones_r[:], ones_f[0:1, :])
            ones65_r = cpool.tile([VW, 128], f32r)
            nc.vector.tensor_copy(ones65_r[DH:DH + 1, :],
                                  ones_f[DH:DH + 1, :])

            # compacted mask (nkv,) int32 -> additive bias tile (128, NMK):
            # adder[p, m] = (maskc[m*128+p] - 1) * 10000
            mask_t = cpool.tile([128, NMK], i32)
            nc.sync.dma_start(
                mask_t[:], mask_d.ap().rearrange("(m p) -> p m", p=128))
            maskf = cpool.tile([128, NMK], f32)
            nc.vector.tensor_copy(maskf[:], mask_t[:])
            adder = cpool.tile([128, NMK], f32)
            nc.vector.tensor_scalar(adder[:], maskf[:], 10000.0, -10000.0,
                                    MULT, ADD)

            # biases: bq/bk as per-partition columns, bv as a 1-row vector
            bq_t = cpool.tile([128, NT], f32)
            nc.sync.dma_start(
                bq_t[:], bq_d.ap().rearrange("(t p) -> p t", p=128))
            bk_t = cpool.tile([128, NT], f32)
            nc.sync.dma_start(
                bk_t[:], bk_d.ap().rearrange("(t p) -> p t", p=128))
            bv_r = cpool.tile([1, HD], f32r)
            nc.sync.dma_start(bv_r[:],
                              bv_d.ap().rearrange("(o n) -> o n", o=1))

            qT = qkpool.tile([128, NT * S], f32r)    # q transposed
            kT = qkpool.tile([128, NT * nkv], f32r)  # k transposed, compacted
            vA = vpool.tile([128, NMK * HPC * VW], f32r)  # v' with ones cols
            nc.vector.tensor_copy(
                vA[:].rearrange("p (m h e) -> p m h e", m=NMK, h=HPC)
                [:, :, :, DH:DH + 1],
                ones_f[:, 0:NMK * HPC].rearrange(
                    "p (m h e) -> p m h e", m=NMK, h=HPC))

            xt_bufs = 16 if nkv < S else 8
            xk_bufs = 12 if nkv < S else 8

            for rep in range(reps):
                # ---------------- QKV projection phase ----------------
                with (
                    tc.tile_pool(name=f"xtp{rep}", bufs=xt_bufs) as xpool,
                    tc.tile_pool(name=f"xkp{rep}", bufs=xk_bufs) as xkpool,
                    tc.tile_pool(name=f"wp{rep}", bufs=1) as wpool,
                    tc.tile_pool(name=f"psq{rep}", bufs=4,
                                 space="PSUM") as psq,
                ):
                    wqt = wpool.tile([128, KD * HD], f32r)
                    wkt = wpool.tile([128, KD * HD], f32r)
                    wvt = wpool.tile([128, KD * HD], f32r)
                    for wt, wd in ((wqt, wq_d), (wkt, wk_d), (wvt, wv_d)):
                        for d in range(KD):
                            nc.sync.dma_start(
                                wt[:, d * HD:(d + 1) * HD],
                                wd.ap()[d * 128:(d + 1) * 128, :])

                    # k, v: compacted sequence
                    for off, w in kv_pieces:
                        xkp = []
                        for d in range(KD):
                            t_ = xkpool.tile([128, 512], f32r, tag="xk")
                            nc.sync.dma_start(
                                t_[:, 0:w],
                                xkt_d.ap()[d * 128:(d + 1) * 128,
                                           off:off + w])
                            xkp.append(t_)
                        for t in range(NT):
                            ps = psq.tile([128, 512], f32, tag="psqkv")
                            for d in range(KD):
                                nc.tensor.matmul(
                                    ps[:, 0:w],
                                    wkt[:, d * HD + t * 128:
                                        d * HD + (t + 1) * 128],
                                    xkp[d][:, 0:w],
                                    start=(d == 0), stop=(d == KD - 1))
                            nc.vector.tensor_scalar_add(
                                kT[:, t * nkv + off:t * nkv + off + w],
                                ps[:, 0:w], bk_t[:, t:t + 1])
                        for mi in range(w // 128):
                            m = off // 128 + mi
                            ps = psq.tile([128, 512], f32, tag="psqkv")
                            for d in range(KD):
                                nc.tensor.matmul(
                                    ps[:],
                                    xkp[d][:, mi * 128:(mi + 1) * 128],
                                    wvt[:, d * HD:(d + 1) * HD],
                                    start=(d == 0), stop=False)
                            nc.tensor.matmul(
                                ps[:], ones_r[:], bv_r[:],
                                start=False, stop=True)
                            nc.vector.tensor_copy(
                                vA[:, m * HPC * VW:(m + 1) * HPC * VW]
                                .rearrange("p (h e) -> p h e",
                                           h=HPC)[:, :, 0:DH],
                                ps[:].rearrange("p (h e) -> p h e", h=HPC))

                    # q: full sequence
                    for c4 in range(NC4):
                        xp = []
                        for d in range(KD):
                            t_ = xpool.tile([128, 512], f32r, tag="xt")
                            nc.sync.dma_start(
                                t_[:],
                                xt_d.ap()[d * 128:(d + 1) * 128,
                                          c4 * 512:(c4 + 1) * 512])
                            xp.append(t_)
                        for t in range(NT):
                            ps = psq.tile([128, 512], f32, tag="psqkv")
                            for d in range(KD):
                                nc.tensor.matmul(
                                    ps[:],
                                    wqt[:, d * HD + t * 128:
                                        d * HD + (t + 1) * 128],
                                    xp[d][:],
                                    start=(d == 0), stop=(d == KD - 1))
                            nc.vector.tensor_scalar_add(
                                qT[:, t * S + c4 * 512:
                                   t * S + (c4 + 1) * 512],
                                ps[:], bq_t[:, t:t + 1])

                if taps:
                    nc.sync.dma_start(dqt_d.ap(), qT[:])
                    nc.sync.dma_start(dkt_d.ap(), kT[:])
                    nc.sync.dma_start(dva_d.ap(), vA[:])

                # ---------------- attention phase ----------------
                with (
                    tc.tile_pool(name=f"probs{rep}", bufs=6) as ppool,
                    tc.tile_pool(name=f"pssc{rep}", bufs=2,
                                 space="PSUM") as pssc,
                    tc.tile_pool(name=f"psctx{rep}", bufs=3,
                                 space="PSUM") as psctx,
                    tc.tile_pool(name=f"psbc{rep}", bufs=1,
                                 space="PSUM") as psbc,
                ):
                    for g in range(NT):        # head pair (qT/kT chunk)
                        for c in range(NC4):   # q chunk of 512
                            ctxA = psctx.tile([VW, 512], f32, tag="ctx")
                            ctxB = psctx.tile([VW, 512], f32, tag="ctx")
                            for m in range(nmk_attn):
                                sc = pssc.tile([128, 1024], f32, tag="sc")
                                # scoresT for the pair, row-packed on the PE
                                nc.tensor.matmul(
                                    sc[:, 0:512],
                                    kT[0:64, g * nkv + m * 128:
                                       g * nkv + (m + 1) * 128],
                                    qT[0:64, g * S + c * 512:
                                       g * S + (c + 1) * 512],
                                    start=True, stop=True)
                                nc.tensor.matmul(
                                    sc[:, 512:1024],
                                    kT[64:128, g * nkv + m * 128:
                                       g * nkv + (m + 1) * 128],
                                    qT[64:128, g * S + c * 512:
                                       g * S + (c + 1) * 512],
                                    start=True, stop=True)
                                probs = ppool.tile([128, 1024], f32r,
                                                   tag="probs")
                                nc.scalar.activation(
                                    probs[:], sc[:], EXP,
                                    bias=adder[:, m:m + 1], scale=0.125)
                                hA, hB = 2 * g, 2 * g + 1
                                nc.tensor.matmul(
                                    ctxA[:],
                                    vA[:, m * HPC * VW + hA * VW:
                                       m * HPC * VW + (hA + 1) * VW],
                                    probs[:, 0:512],
                                    start=(m == 0), stop=(m == nmk_attn - 1))
                                nc.tensor.matmul(
                                    ctxB[:],
                                    vA[:, m * HPC * VW + hB * VW:
                                       m * HPC * VW + (hB + 1) * VW],
                                    probs[:, 512:1024],
                                    start=(m == 0), stop=(m == nmk_attn - 1))

                            for h, ctx in ((2 * g, ctxA), (2 * g + 1, ctxB)):
                                # raw sums row (partition 64) -> SBUF, then
                                # broadcast to partitions 0..63 via a K=1
                                # matmul (the PE handles base-64 operands;
                                # the custom DVE recip reads partition 0),
                                # then reciprocal on the base-0 tile.
                                sums_r = rcpool.tile([VW, 512], f32r,
                                                     tag="sums")
                                nc.vector.tensor_copy(
                                    sums_r[DH:DH + 1, :], ctx[DH:DH + 1, :])
                                bc = psbc.tile([DH, 512], f32, tag="bc")
                                nc.tensor.matmul(
                                    bc[:], ones65_r[DH:DH + 1, 0:DH],
                                    sums_r[DH:DH + 1, :],
                                    start=True, stop=True)
                                bcs = bcpool.tile([DH, 512], f32, tag="bcs")
                                nc.vector.reciprocal_approx_fast(
                                    out=bcs[:], in_=bc[:])
                                o = opool.tile([DH, 512], f32, tag="o")
                                nc.vector.tensor_mul(o[:], ctx[0:DH, :],
                                                     bcs[:])
                                nc.sync.dma_start(
                                    out_d.ap()[h * DH:(h + 1) * DH,
                                               c * 512:(c + 1) * 512], o[:])

    nc.compile()
    return nc


def get_nc(nkv, nmk_attn):
    key = (nkv, nmk_attn)
    if key not in _CACHED:
        _CACHED[key] = _build_nc(nkv, nmk_attn)
    return _CACHED[key]


def make_in_maps(nkv, x, mask, wq, bq, wk, bk, wv, bv):
    x = np.ascontiguousarray(np.asarray(x, dtype=np.float32))
    mask = np.ascontiguousarray(np.asarray(mask, dtype=np.int32))
    wq = np.asarray(wq, dtype=np.float32)
    wk = np.asarray(wk, dtype=np.float32)
    wv = np.asarray(wv, dtype=np.float32)
    bq = np.asarray(bq, dtype=np.float32)
    bk = np.asarray(bk, dtype=np.float32)
    bv = np.asarray(bv, dtype=np.float32)
    # per-batch kv compaction indices (unmasked first, masked as padding)
    idx = []
    for b in range(B):
        on = np.flatnonzero(mask[b] != 0)
        off = np.flatnonzero(mask[b] == 0)
        ib = np.concatenate([on, off])[:nkv]
        idx.append(ib)
    in_maps = []
    for c in range(NCORES):
        b, g = c // 2, c % 2
        cols = slice(g * HD, (g + 1) * HD)
        xtb = np.ascontiguousarray(x[b].T)
        in_maps.append({
            "xt": xtb,
            "xkt": np.ascontiguousarray(xtb[:, idx[b]]),
            "wq": np.ascontiguousarray(wq[:, cols]),
            "wk": np.ascontiguousarray(wk[:, cols]),
            "wv": np.ascontiguousarray(wv[:, cols]),
            "bq": np.ascontiguousarray(bq[cols]),
            "bk": np.ascontiguousarray(bk[cols]),
            "bv": np.ascontiguousarray(bv[cols]),
            "maskc": np.ascontiguousarray(mask[b][idx[b]]),
        })
    return in_maps


def assemble_out(results):
    out = np.empty((B, S, H * DH), dtype=np.float32)
    for c in range(NCORES):
        b, g = c // 2, c % 2
        out[b, :, g * HD:(g + 1) * HD] = results[c]["out"].T
    return out


def pick_nkv(mask):
    mask = np.asarray(mask)
    nb_max = int((mask != 0).sum(axis=1).max())
    nmk_attn = max(1, -(-nb_max // 128))
    nkv = min(-(-(nmk_attn * 128) // 512) * 512, S)
    return nkv, nmk_attn


def run(trace=False, **inputs):
    from concourse.bass_utils import run_bass_kernel_spmd

    nkv, nmk_attn = pick_nkv(inputs["mask"])
    nc = get_nc(nkv, nmk_attn)
    in_maps = make_in_maps(nkv, **inputs)
    res = run_bass_kernel_spmd(nc, in_maps, core_ids=list(range(NCORES)),
                               trace=trace)
    return assemble_out(res.results), res


def kernel(**inputs):
    out, _ = run(trace=False, **inputs)
    return out


# revision 20
# speedup vs baseline: 1.3488x; 1.3440x over previous
"""Multi-head attention layer (B=4, S=2048, D=1024, H=16, DH=64) on 8 TRN2 cores.

Sharding: core c = (batch b, head-group g) with b = c//2, g = c%2.
Each core computes QKV projections for one batch with an 8-head column slice
of the weights, then full attention for those 8 heads — zero collectives.

Per-core layout choices:
  - x is passed host-transposed as xT (D, S) so the D-contraction sits on
    SBUF partitions for all three projection matmuls without device transposes.
  - key/value sequence compaction: masked key positions contribute exactly 0
    to the softmax (exp(-10000 + s) == 0 in f32), so the host gathers the
    unmasked positions (padded with masked ones to a static NKV) into a
    separate compacted operand xkT used for the K/V projections. Scores, exp
    and context shrink by NKV/S. NKV=1280 covers any mask with <=1280 active
    keys (a +11 sigma event for the Bernoulli(0.5) mask); kernel() falls back
    to an NKV=2048 graph otherwise, which is correct for any mask.
  - q, k are produced transposed (qT/kT: head_dim on partitions, seq free) so
    the scores matmul contracts over DH=64 directly; two heads share the PE
    array via row tile_position packing (base partitions 0 / 64).
  - scores are computed transposed (k on partitions, q free); the mask adder
    is a per-partition bias and the softmax exp is a single ScalarE
    activation (exp(0.125*s + adder)) straight out of PSUM.
  - v is produced in natural layout (kv position on partitions) with a
    ones-column appended per head, so the context matmul (M=65) yields the
    softmax denominators in PSUM row 64 for free.
  - normalization: the denominator row is broadcast across partitions with a
    K=1 outer-product matmul, reciprocal'd on DVE, and applied during the
    PSUM evict multiply. Output is written as ctxT (HD, S); host transposes.

All matmuls run as float32r (full-rate fp32 storage on the PE array).

`reps` repeats the whole compute body inside one NEFF (used only for
device-time measurement: wall(reps=k) - wall(reps=1) isolates body time
from host/RPC overhead).
"""

import os
import sys

import numpy as np

sys.path.insert(0, "/opt/trn_rl_repo")

B, S, D = 4, 2048, 1024
H, DH = 16, 64
HPC = 8            # heads per core
HD = HPC * DH      # 512: output columns per core
NCORES = 8
KD = D // 128      # 8 contraction chunks
NT = HD // 128     # 4 head-dim partition chunks (= head pairs)
NC4 = S // 512     # 4 q chunks of 512
VW = DH + 1        # 65: v columns per head incl. ones column

_CACHED = {}


def _pieces(n):
    """Split n into pieces of 512 with any tail split so every piece is a
    multiple of 128 and >= 256 (f32r matmuls need a moving dim >= 256 for
    full rate)."""
    out, off = [], 0
    while n - off > 768:
        out.append((off, 512))
        off += 512
    rem = n - off
    if rem > 512:
        out.append((off, rem - 256))
        out.append((off + rem - 256, 256))
    elif rem:
        out.append((off, rem))
    assert all(w >= 256 and w % 128 == 0 for _, w in out), out
    return out


def _build_nc(nkv, nmk_attn=None, taps=False, reps=1):
    from concourse import bacc, mybir, tile

    f32 = mybir.dt.float32
    f32r = mybir.dt.float32r
    i32 = mybir.dt.int32
    EXP = mybir.ActivationFunctionType.Exp
    MULT = mybir.AluOpType.mult
    ADD = mybir.AluOpType.add

    NMK = nkv // 128          # kv chunks of 128 (projection width)
    if nmk_attn is None:
        nmk_attn = NMK        # attention window in 128-chunks
    assert nmk_attn <= NMK
    kv_pieces = _pieces(nkv)

    nc = bacc.Bacc("TRN2", target_bir_lowering=False, debug=False,
                   enable_asserts=False)

    xt_d = nc.declare_dram_parameter("xt", [D, S], f32r, isOutput=False)
    xkt_d = nc.declare_dram_parameter("xkt", [D, nkv], f32r, isOutput=False)
    wq_d = nc.declare_dram_parameter("wq", [D, HD], f32r, isOutput=False)
    wk_d = nc.declare_dram_parameter("wk", [D, HD], f32r, isOutput=False)
    wv_d = nc.declare_dram_parameter("wv", [D, HD], f32r, isOutput=False)
    bq_d = nc.declare_dram_parameter("bq", [HD], f32, isOutput=False)
    bk_d = nc.declare_dram_parameter("bk", [HD], f32, isOutput=False)
    bv_d = nc.declare_dram_parameter("bv", [HD], f32r, isOutput=False)
    mask_d = nc.declare_dram_parameter("maskc", [nkv], i32, isOutput=False)
    out_d = nc.declare_dram_parameter("out", [HD, S], f32, isOutput=True)
    if taps:
        dqt_d = nc.declare_dram_parameter("dqt", [128, NT * S], f32r,
                                          isOutput=True)
        dkt_d = nc.declare_dram_parameter("dkt", [128, NT * nkv], f32r,
                                          isOutput=True)
        dva_d = nc.declare_dram_parameter("dva", [128, NMK * HPC * VW], f32r,
                                          isOutput=True)

    with tile.TileContext(nc) as tc:
        with (
            tc.tile_pool(name="const", bufs=1) as cpool,
            tc.tile_pool(name="qk", bufs=1) as qkpool,
            tc.tile_pool(name="vv", bufs=1) as vpool,
            tc.tile_pool(name="outp", bufs=4) as opool,
            tc.tile_pool(name="bcp", bufs=2) as bcpool,
            tc.tile_pool(name="rcp", bufs=3) as rcpool,
        ):
            ones_f = cpool.tile([128, 128], f32)
            nc.vector.memset(ones_f[:], 1.0)
            ones_r = cpool.tile([1, 128], f32r)
            nc.vector.tensor_copy(ones_r[:], ones_f[0:1, :])
            ones65_r = cpool.tile([VW, 128], f32r)
            nc.vector.tensor_copy(ones65_r[DH:DH + 1, :],
                                  ones_f[DH:DH + 1, :])

            # compacted mask (nkv,) int32 -> additive bias tile (128, NMK):
            # adder[p, m] = (maskc[m*128+p] - 1) * 10000
            mask_t = cpool.tile([128, NMK], i32)
            nc.sync.dma_start(
                mask_t[:], mask_d.ap().rearrange("(m p) -> p m", p=128))
            maskf = cpool.tile([128, NMK], f32)
            nc.vector.tensor_copy(maskf[:], mask_t[:])
            adder = cpool.tile([128, NMK], f32)
            nc.vector.tensor_scalar(adder[:], maskf[:], 10000.0, -10000.0,
                                    MULT, ADD)

            # biases: bq/bk as per-partition columns, bv as a 1-row vector
            bq_t = cpool.tile([128, NT], f32)
            nc.sync.dma_start(
                bq_t[:], bq_d.ap().rearrange("(t p) -> p t", p=128))
            bk_t = cpool.tile([128, NT], f32)
            nc.sync.dma_start(
                bk_t[:], bk_d.ap().rearrange("(t p) -> p t", p=128))
            bv_r = cpool.tile([1, HD], f32r)
            nc.sync.dma_start(bv_r[:],
                              bv_d.ap().rearrange("(o n) -> o n", o=1))

            qT = qkpool.tile([128, NT * S], f32r)    # q transposed
            kT = qkpool.tile([128, NT * nkv], f32r)  # k transposed, compacted
            vA = vpool.tile([128, NMK * HPC * VW], f32r)  # v' with ones cols
            nc.vector.tensor_copy(
                vA[:].rearrange("p (m h e) -> p m h e", m=NMK, h=HPC)
                [:, :, :, DH:DH + 1],
                ones_f[:, 0:NMK * HPC].rearrange(
                    "p (m h e) -> p m h e", m=NMK, h=HPC))

            xt_bufs = 16 if nkv < S else 8
            xk_bufs = 12 if nkv < S else 8

            for rep in range(reps):
                # ---------------- QKV projection phase ----------------
                with (
                    tc.tile_pool(name=f"xtp{rep}", bufs=xt_bufs) as xpool,
                    tc.tile_pool(name=f"xkp{rep}", bufs=xk_bufs) as xkpool,
                    tc.tile_pool(name=f"wp{rep}", bufs=1) as wpool,
                    tc.tile_pool(name=f"psq{rep}", bufs=4,
                                 space="PSUM") as psq,
                ):
                    wqt = wpool.tile([128, KD * HD], f32r)
                    wkt = wpool.tile([128, KD * HD], f32r)
                    wvt = wpool.tile([128, KD * HD], f32r)

                    def dma_w(wt, wd):
                        for d in range(KD):
                            nc.sync.dma_start(
                                wt[:, d * HD:(d + 1) * HD],
                                wd.ap()[d * 128:(d + 1) * 128, :])
                    # consumption order: wk gates the very first matmul,
                    # then the first xk piece (emitted in the piece loop),
                    # then wv (v-proj of piece 0), wq last before the q loop.
                    dma_w(wkt, wk_d)

                    # k, v: compacted sequence
                    for pi, (off, w) in enumerate(kv_pieces):
                        xkp = []
                        for d in range(KD):
                            t_ = xkpool.tile([128, 512], f32r, tag="xk")
                            nc.sync.dma_start(
                                t_[:, 0:w],
                                xkt_d.ap()[d * 128:(d + 1) * 128,
                                           off:off + w])
                            xkp.append(t_)
                        if pi == 0:
                            dma_w(wvt, wv_d)
                        for t in range(NT):
                            ps = psq.tile([128, 512], f32, tag="psqkv")
                            for d in range(KD):
                                nc.tensor.matmul(
                                    ps[:, 0:w],
                                    wkt[:, d * HD + t * 128:
                                        d * HD + (t + 1) * 128],
                                    xkp[d][:, 0:w],
                                    start=(d == 0), stop=(d == KD - 1))
                            nc.vector.tensor_scalar_add(
                                kT[:, t * nkv + off:t * nkv + off + w],
                                ps[:, 0:w], bk_t[:, t:t + 1])
                        for mi in range(w // 128):
                            m = off // 128 + mi
                            ps = psq.tile([128, 512], f32, tag="psqkv")
                            for d in range(KD):
                                nc.tensor.matmul(
                                    ps[:],
                                    xkp[d][:, mi * 128:(mi + 1) * 128],
                                    wvt[:, d * HD:(d + 1) * HD],
                                    start=(d == 0), stop=False)
                            nc.tensor.matmul(
                                ps[:], ones_r[:], bv_r[:],
                                start=False, stop=True)
                            nc.vector.tensor_copy(
                                vA[:, m * HPC * VW:(m + 1) * HPC * VW]
                                .rearrange("p (h e) -> p h e",
                                           h=HPC)[:, :, 0:DH],
                                ps[:].rearrange("p (h e) -> p h e", h=HPC))

                    # q: full sequence
                    dma_w(wqt, wq_d)
                    for c4 in range(NC4):
                        xp = []
                        for d in range(KD):
                            t_ = xpool.tile([128, 512], f32r, tag="xt")
                            nc.sync.dma_start(
                                t_[:],
                                xt_d.ap()[d * 128:(d + 1) * 128,
                                          c4 * 512:(c4 + 1) * 512])
                            xp.append(t_)
                        for t in range(NT):
                            ps = psq.tile([128, 512], f32, tag="psqkv")
                            for d in range(KD):
                                nc.tensor.matmul(
                                    ps[:],
                                    wqt[:, d * HD + t * 128:
                                        d * HD + (t + 1) * 128],
                                    xp[d][:],
                                    start=(d == 0), stop=(d == KD - 1))
                            nc.vector.tensor_scalar_add(
                                qT[:, t * S + c4 * 512:
                                   t * S + (c4 + 1) * 512],
                                ps[:], bq_t[:, t:t + 1])

                if taps:
                    nc.sync.dma_start(dqt_d.ap(), qT[:])
                    nc.sync.dma_start(dkt_d.ap(), kT[:])
                    nc.sync.dma_start(dva_d.ap(), vA[:])

                # ---------------- attention phase ----------------
                with (
                    tc.tile_pool(name=f"probs{rep}", bufs=6) as ppool,
                    tc.tile_pool(name=f"pssc{rep}", bufs=2,
                                 space="PSUM") as pssc,
                    tc.tile_pool(name=f"psctx{rep}", bufs=3,
                                 space="PSUM") as psctx,
                    tc.tile_pool(name=f"psbc{rep}", bufs=1,
                                 space="PSUM") as psbc,
                ):
                    for g in range(NT):        # head pair (qT/kT chunk)
                        for c in range(NC4):   # q chunk of 512
                            ctxA = psctx.tile([VW, 512], f32, tag="ctx")
                            ctxB = psctx.tile([VW, 512], f32, tag="ctx")
                            for m in range(nmk_attn):
                                sc = pssc.tile([128, 1024], f32, tag="sc")
                                # scoresT for the pair, row-packed on the PE
                                nc.tensor.matmul(
                                    sc[:, 0:512],
                                    kT[0:64, g * nkv + m * 128:
                                       g * nkv + (m + 1) * 128],
                                    qT[0:64, g * S + c * 512:
                                       g * S + (c + 1) * 512],
                                    start=True, stop=True)
                                nc.tensor.matmul(
                                    sc[:, 512:1024],
                                    kT[64:128, g * nkv + m * 128:
                                       g * nkv + (m + 1) * 128],
                                    qT[64:128, g * S + c * 512:
                                       g * S + (c + 1) * 512],
                                    start=True, stop=True)
                                probs = ppool.tile([128, 1024], f32r,
                                                   tag="probs")
                                nc.scalar.activation(
                                    probs[:], sc[:], EXP,
                                    bias=adder[:, m:m + 1], scale=0.125)
                                hA, hB = 2 * g, 2 * g + 1
                                nc.tensor.matmul(
                                    ctxA[:],
                                    vA[:, m * HPC * VW + hA * VW:
                                       m * HPC * VW + (hA + 1) * VW],
                                    probs[:, 0:512],
                                    start=(m == 0), stop=(m == nmk_attn - 1))
                                nc.tensor.matmul(
                                    ctxB[:],
                                    vA[:, m * HPC * VW + hB * VW:
                                       m * HPC * VW + (hB + 1) * VW],
                                    probs[:, 512:1024],
                                    start=(m == 0), stop=(m == nmk_attn - 1))

                            for h, ctx in ((2 * g, ctxA), (2 * g + 1, ctxB)):
                                # single evict frees the ctx PSUM bank fast;
                                # normalization then runs from SBUF: a K=1
                                # matmul broadcasts the raw sums row to
                                # partitions 0..63 (the PE handles base-64
                                # operands; the custom DVE recip reads
                                # partition 0), reciprocal, multiply, DMA.
                                u = rcpool.tile([VW, 512], f32r, tag="u")
                                nc.vector.tensor_copy(u[:], ctx[:])
                                bc = psbc.tile([DH, 512], f32, tag="bc")
                                nc.tensor.matmul(
                                    bc[:], ones65_r[DH:DH + 1, 0:DH],
                                    u[DH:DH + 1, :],
                                    start=True, stop=True)
                                bcs = bcpool.tile([DH, 512], f32, tag="bcs")
                                nc.vector.reciprocal_approx_fast(
                                    out=bcs[:], in_=bc[:])
                                o = opool.tile([DH, 512], f32, tag="o")
                                nc.vector.tensor_mul(o[:], u[0:DH, :],
                                                     bcs[:])
                                nc.sync.dma_start(
                                    out_d.ap()[h * DH:(h + 1) * DH,
                                               c * 512:(c + 1) * 512], o[:])

    nc.compile()
    return nc


def get_nc(nkv, nmk_attn):
    key = (nkv, nmk_attn)
    if key not in _CACHED:
        _CACHED[key] = _build_nc(nkv, nmk_attn)
    return _CACHED[key]


def make_in_maps(nkv, x, mask, wq, bq, wk, bk, wv, bv):
    x = np.ascontiguousarray(np.asarray(x, dtype=np.float32))
    mask = np.ascontiguousarray(np.asarray(mask, dtype=np.int32))
    wq = np.asarray(wq, dtype=np.float32)
    wk = np.asarray(wk, dtype=np.float32)
    wv = np.asarray(wv, dtype=np.float32)
    bq = np.asarray(bq, dtype=np.float32)
    bk = np.asarray(bk, dtype=np.float32)
    bv = np.asarray(bv, dtype=np.float32)
    # per-batch kv compaction indices (unmasked first, masked as padding)
    idx = []
    for b in range(B):
        on = np.flatnonzero(mask[b] != 0)
        off = np.flatnonzero(mask[b] == 0)
        ib = np.concatenate([on, off])[:nkv]
        idx.append(ib)
    in_maps = []
    for c in range(NCORES):
        b, g = c // 2, c % 2
        cols = slice(g * HD, (g + 1) * HD)
        xtb = np.ascontiguousarray(x[b].T)
        in_maps.append({
            "xt": xtb,
            "xkt": np.ascontiguousarray(xtb[:, idx[b]]),
            "wq": np.ascontiguousarray(wq[:, cols]),
            "wk": np.ascontiguousarray(wk[:, cols]),
            "wv": np.ascontiguousarray(wv[:, cols]),
            "bq": np.ascontiguousarray(bq[cols]),
            "bk": np.ascontiguousarray(bk[cols]),
            "bv": np.ascontiguousarray(bv[cols]),
            "maskc": np.ascontiguousarray(mask[b][idx[b]]),
        })
    return in_maps


def assemble_out(results):
    out = np.empty((B, S, H * DH), dtype=np.float32)
    for c in range(NCORES):
        b, g = c // 2, c % 2
        out[b, :, g * HD:(g + 1) * HD] = results[c]["out"].T
    return out


def pick_nkv(mask):
    mask = np.asarray(mask)
    nb_max = int((mask != 0).sum(axis=1).max())
    nmk_attn = max(1, -(-nb_max // 128))
    nkv = min(-(-(nmk_attn * 128) // 512) * 512, S)
    return nkv, nmk_attn


def run(trace=False, **inputs):
    from concourse.bass_utils import run_bass_kernel_spmd

    nkv, nmk_attn = pick_nkv(inputs["mask"])
    nc = get_nc(nkv, nmk_attn)
    in_maps = make_in_maps(nkv, **inputs)
    res = run_bass_kernel_spmd(nc, in_maps, core_ids=list(range(NCORES)),
                               trace=trace)
    return assemble_out(res.results), res


def kernel(**inputs):
    out, _ = run(trace=False, **inputs)
    return out


# revision 22
# speedup vs baseline: 1.5915x; 1.1800x over previous
"""Multi-head attention layer (B=4, S=2048, D=1024, H=16, DH=64) on 8 TRN2 cores.

Sharding: core c = (batch b, head-group g) with b = c//2, g = c%2.
Each core computes QKV projections for one batch with an 8-head column slice
of the weights, then full attention for those 8 heads — zero collectives.

Per-core layout choices:
  - x is passed host-transposed as xT (D, S) so the D-contraction sits on
    SBUF partitions for all three projection matmuls without device transposes.
  - key/value sequence compaction: masked key positions contribute exactly 0
    to the softmax (exp(-10000 + s) == 0 in f32), so the host gathers the
    unmasked positions (padded with masked ones to a static NKV) into a
    separate compacted operand xkT used for the K/V projections. Scores, exp
    and context shrink by NKV/S. NKV=1280 covers any mask with <=1280 active
    keys (a +11 sigma event for the Bernoulli(0.5) mask); kernel() falls back
    to an NKV=2048 graph otherwise, which is correct for any mask.
  - q, k are produced transposed (qT/kT: head_dim on partitions, seq free) so
    the scores matmul contracts over DH=64 directly; two heads share the PE
    array via row tile_position packing (base partitions 0 / 64).
  - scores are computed transposed (k on partitions, q free); the mask adder
    is a per-partition bias and the softmax exp is a single ScalarE
    activation (exp(0.125*s + adder)) straight out of PSUM.
  - v is produced in natural layout (kv position on partitions) with a
    ones-column appended per head, so the context matmul (M=65) yields the
    softmax denominators in PSUM row 64 for free.
  - normalization: the denominator row is broadcast across partitions with a
    K=1 outer-product matmul, reciprocal'd on DVE, and applied during the
    PSUM evict multiply. Output is written as ctxT (HD, S); host transposes.

All matmuls run as float32r (full-rate fp32 storage on the PE array).

`reps` repeats the whole compute body inside one NEFF (used only for
device-time measurement: wall(reps=k) - wall(reps=1) isolates body time
from host/RPC overhead).
"""

import os
import sys

import numpy as np

sys.path.insert(0, "/opt/trn_rl_repo")

B, S, D = 4, 2048, 1024
H, DH = 16, 64
HPC = 8            # heads per core
HD = HPC * DH      # 512: output columns per core
NCORES = 8
KD = D // 128      # 8 contraction chunks
NT = HD // 128     # 4 head-dim partition chunks (= head pairs)
NC4 = S // 512     # 4 q chunks of 512
VW = DH + 1        # 65: v columns per head incl. ones column

_CACHED = {}


def _pieces(n):
    """Split n into pieces of 512 with any tail split so every piece is a
    multiple of 128 and >= 256 (f32r matmuls need a moving dim >= 256 for
    full rate)."""
    out, off = [], 0
    while n - off > 768:
        out.append((off, 512))
        off += 512
    rem = n - off
    if rem > 512:
        out.append((off, rem - 256))
        out.append((off + rem - 256, 256))
    elif rem:
        out.append((off, rem))
    assert all(w >= 256 and w % 128 == 0 for _, w in out), out
    return out


def _build_nc(nkv, nmk_attn=None, taps=False, reps=1, has_bv=True):
    from concourse import bacc, mybir, tile

    f32 = mybir.dt.float32
    f32r = mybir.dt.float32r
    i32 = mybir.dt.int32
    EXP = mybir.ActivationFunctionType.Exp
    MULT = mybir.AluOpType.mult
    ADD = mybir.AluOpType.add

    NMK = nkv // 128          # kv chunks of 128 (projection width)
    if nmk_attn is None:
        nmk_attn = NMK        # attention window in 128-chunks
    assert nmk_attn <= NMK
    kv_pieces = _pieces(nkv)

    nc = bacc.Bacc("TRN2", target_bir_lowering=False, debug=False,
                   enable_asserts=False)

    xt_d = nc.declare_dram_parameter("xt", [D, S], f32r, isOutput=False)
    xkt_d = nc.declare_dram_parameter("xkt", [D, nkv], f32r, isOutput=False)
    wq_d = nc.declare_dram_parameter("wq", [D, HD], f32r, isOutput=False)
    wk_d = nc.declare_dram_parameter("wk", [D, HD], f32r, isOutput=False)
    wv_d = nc.declare_dram_parameter("wv", [D, HD], f32r, isOutput=False)
    bq_d = nc.declare_dram_parameter("bq", [HD], f32, isOutput=False)
    bk_d = nc.declare_dram_parameter("bk", [HD], f32, isOutput=False)
    bv_d = (nc.declare_dram_parameter("bv", [HD], f32r, isOutput=False)
            if has_bv else None)
    mask_d = nc.declare_dram_parameter("maskc", [nkv], i32, isOutput=False)
    out_d = nc.declare_dram_parameter("out", [HD, S], f32, isOutput=True)
    if taps:
        dqt_d = nc.declare_dram_parameter("dqt", [128, NT * S], f32r,
                                          isOutput=True)
        dkt_d = nc.declare_dram_parameter("dkt", [128, NT * nkv], f32r,
                                          isOutput=True)
        dva_d = nc.declare_dram_parameter("dva", [128, NMK * HPC * VW], f32r,
                                          isOutput=True)

    with tile.TileContext(nc) as tc:
        with (
            tc.tile_pool(name="const", bufs=1) as cpool,
            tc.tile_pool(name="qk", bufs=1) as qkpool,
            tc.tile_pool(name="vv", bufs=1) as vpool,
            tc.tile_pool(name="outp", bufs=4) as opool,
            tc.tile_pool(name="bcp", bufs=2) as bcpool,
            tc.tile_pool(name="rcp", bufs=3) as rcpool,
        ):
            ones_f = cpool.tile([128, 128], f32)
            nc.vector.memset(ones_f[:], 1.0)
            ones_r = cpool.tile([1, 128], f32r)
            nc.vector.tensor_copy(ones_r[:], ones_f[0:1, :])
            ones65_r = cpool.tile([VW, 128], f32r)
            nc.vector.tensor_copy(ones65_r[DH:DH + 1, :],
                                  ones_f[DH:DH + 1, :])

            # compacted mask (nkv,) int32 -> additive bias tile (128, NMK):
            # adder[p, m] = (maskc[m*128+p] - 1) * 10000
            mask_t = cpool.tile([128, NMK], i32)
            nc.sync.dma_start(
                mask_t[:], mask_d.ap().rearrange("(m p) -> p m", p=128))
            maskf = cpool.tile([128, NMK], f32)
            nc.vector.tensor_copy(maskf[:], mask_t[:])
            adder = cpool.tile([128, NMK], f32)
            nc.vector.tensor_scalar(adder[:], maskf[:], 10000.0, -10000.0,
                                    MULT, ADD)

            # biases: bq/bk as per-partition columns, bv as a 1-row vector
            bq_t = cpool.tile([128, NT], f32)
            nc.sync.dma_start(
                bq_t[:], bq_d.ap().rearrange("(t p) -> p t", p=128))
            bk_t = cpool.tile([128, NT], f32)
            nc.sync.dma_start(
                bk_t[:], bk_d.ap().rearrange("(t p) -> p t", p=128))
            if has_bv:
                bv_r = cpool.tile([1, HD], f32r)
                nc.sync.dma_start(bv_r[:],
                                  bv_d.ap().rearrange("(o n) -> o n", o=1))

            qT = qkpool.tile([128, NT * S], f32r)    # q transposed
            kT = qkpool.tile([128, NT * nkv], f32r)  # k transposed, compacted
            vA = vpool.tile([128, NMK * HPC * VW], f32r)  # v' with ones cols
            nc.vector.tensor_copy(
                vA[:].rearrange("p (m h e) -> p m h e", m=NMK, h=HPC)
                [:, :, :, DH:DH + 1],
                ones_f[:, 0:NMK * HPC].rearrange(
                    "p (m h e) -> p m h e", m=NMK, h=HPC))

            xt_bufs = 16 if nkv < S else 8
            xk_bufs = 12 if nkv < S else 8

            for rep in range(reps):
                # ---------------- QKV projection phase ----------------
                with (
                    tc.tile_pool(name=f"xtp{rep}", bufs=xt_bufs) as xpool,
                    tc.tile_pool(name=f"xkp{rep}", bufs=xk_bufs) as xkpool,
                    tc.tile_pool(name=f"wp{rep}", bufs=1) as wpool,
                    tc.tile_pool(name=f"psq{rep}", bufs=4,
                                 space="PSUM") as psq,
                ):
                    wqt = wpool.tile([128, KD * HD], f32r)
                    wkt = wpool.tile([128, KD * HD], f32r)
                    wvt = wpool.tile([128, KD * HD], f32r)

                    def dma_w(wt, wd):
                        for d in range(KD):
                            nc.sync.dma_start(
                                wt[:, d * HD:(d + 1) * HD],
                                wd.ap()[d * 128:(d + 1) * 128, :])
                    # consumption order: the first k matmul needs wk[d0]
                    # AND the first xk chunk of d0 — interleave their
                    # emissions so round-robin queue assignment transfers
                    # them in parallel; then wv (v-proj of piece 0), wq
                    # last before the q loop.
                    # k, v: compacted sequence
                    for pi, (off, w) in enumerate(kv_pieces):
                        xkp = []
                        for d in range(KD):
                            if pi == 0:
                                nc.sync.dma_start(
                                    wkt[:, d * HD:(d + 1) * HD],
                                    wk_d.ap()[d * 128:(d + 1) * 128, :])
                            t_ = xkpool.tile([128, 512], f32r, tag="xk")
                            nc.sync.dma_start(
                                t_[:, 0:w],
                                xkt_d.ap()[d * 128:(d + 1) * 128,
                                           off:off + w])
                            xkp.append(t_)
                        if pi == 0:
                            dma_w(wvt, wv_d)
                        for t in range(NT):
                            ps = psq.tile([128, 512], f32, tag="psqkv")
                            for d in range(KD):
                                nc.tensor.matmul(
                                    ps[:, 0:w],
                                    wkt[:, d * HD + t * 128:
                                        d * HD + (t + 1) * 128],
                                    xkp[d][:, 0:w],
                                    start=(d == 0), stop=(d == KD - 1))
                            nc.vector.tensor_scalar_add(
                                kT[:, t * nkv + off:t * nkv + off + w],
                                ps[:, 0:w], bk_t[:, t:t + 1])
                        for mi in range(w // 128):
                            m = off // 128 + mi
                            ps = psq.tile([128, 512], f32, tag="psqkv")
                            for d in range(KD):
                                nc.tensor.matmul(
                                    ps[:],
                                    xkp[d][:, mi * 128:(mi + 1) * 128],
                                    wvt[:, d * HD:(d + 1) * HD],
                                    start=(d == 0),
                                    stop=(not has_bv and d == KD - 1))
                            if has_bv:
                                nc.tensor.matmul(
                                    ps[:], ones_r[:], bv_r[:],
                                    start=False, stop=True)
                            nc.vector.tensor_copy(
                                vA[:, m * HPC * VW:(m + 1) * HPC * VW]
                                .rearrange("p (h e) -> p h e",
                                           h=HPC)[:, :, 0:DH],
                                ps[:].rearrange("p (h e) -> p h e", h=HPC))

                    # q: full sequence
                    dma_w(wqt, wq_d)
                    for c4 in range(NC4):
                        xp = []
                        for d in range(KD):
                            t_ = xpool.tile([128, 512], f32r, tag="xt")
                            nc.sync.dma_start(
                                t_[:],
                                xt_d.ap()[d * 128:(d + 1) * 128,
                                          c4 * 512:(c4 + 1) * 512])
                            xp.append(t_)
                        for t in range(NT):
                            ps = psq.tile([128, 512], f32, tag="psqkv")
                            for d in range(KD):
                                nc.tensor.matmul(
                                    ps[:],
                                    wqt[:, d * HD + t * 128:
                                        d * HD + (t + 1) * 128],
                                    xp[d][:],
                                    start=(d == 0), stop=(d == KD - 1))
                            nc.vector.tensor_scalar_add(
                                qT[:, t * S + c4 * 512:
                                   t * S + (c4 + 1) * 512],
                                ps[:], bq_t[:, t:t + 1])

                if taps:
                    nc.sync.dma_start(dqt_d.ap(), qT[:])
                    nc.sync.dma_start(dkt_d.ap(), kT[:])
                    nc.sync.dma_start(dva_d.ap(), vA[:])

                # ---------------- attention phase ----------------
                with (
                    tc.tile_pool(name=f"probs{rep}", bufs=6) as ppool,
                    tc.tile_pool(name=f"pssc{rep}", bufs=2,
                                 space="PSUM") as pssc,
                    tc.tile_pool(name=f"psctx{rep}", bufs=3,
                                 space="PSUM") as psctx,
                    tc.tile_pool(name=f"psbc{rep}", bufs=1,
                                 space="PSUM") as psbc,
                ):
                    for g in range(NT):        # head pair (qT/kT chunk)
                        for c in range(NC4):   # q chunk of 512
                            ctxA = psctx.tile([VW, 512], f32, tag="ctx")
                            ctxB = psctx.tile([VW, 512], f32, tag="ctx")
                            for m in range(nmk_attn):
                                sc = pssc.tile([128, 1024], f32, tag="sc")
                                # scoresT for the pair, row-packed on the PE
                                nc.tensor.matmul(
                                    sc[:, 0:512],
                                    kT[0:64, g * nkv + m * 128:
                                       g * nkv + (m + 1) * 128],
                                    qT[0:64, g * S + c * 512:
                                       g * S + (c + 1) * 512],
                                    start=True, stop=True)
                                nc.tensor.matmul(
                                    sc[:, 512:1024],
                                    kT[64:128, g * nkv + m * 128:
                                       g * nkv + (m + 1) * 128],
                                    qT[64:128, g * S + c * 512:
                                       g * S + (c + 1) * 512],
                                    start=True, stop=True)
                                probs = ppool.tile([128, 1024], f32r,
                                                   tag="probs")
                                nc.scalar.activation(
                                    probs[:], sc[:], EXP,
                                    bias=adder[:, m:m + 1], scale=0.125)
                                hA, hB = 2 * g, 2 * g + 1
                                nc.tensor.matmul(
                                    ctxA[:],
                                    vA[:, m * HPC * VW + hA * VW:
                                       m * HPC * VW + (hA + 1) * VW],
                                    probs[:, 0:512],
                                    start=(m == 0), stop=(m == nmk_attn - 1))
                                nc.tensor.matmul(
                                    ctxB[:],
                                    vA[:, m * HPC * VW + hB * VW:
                                       m * HPC * VW + (hB + 1) * VW],
                                    probs[:, 512:1024],
                                    start=(m == 0), stop=(m == nmk_attn - 1))

                            for h, ctx in ((2 * g, ctxA), (2 * g + 1, ctxB)):
                                # single evict frees the ctx PSUM bank fast;
                                # normalization then runs from SBUF: a K=1
                                # matmul broadcasts the raw sums row to
                                # partitions 0..63 (the PE handles base-64
                                # operands; the custom DVE recip reads
                                # partition 0), reciprocal, multiply, DMA.
                                u = rcpool.tile([VW, 512], f32r, tag="u")
                                nc.vector.tensor_copy(u[:], ctx[:])
                                bc = psbc.tile([DH, 512], f32, tag="bc")
                                nc.tensor.matmul(
                                    bc[:], ones65_r[DH:DH + 1, 0:DH],
                                    u[DH:DH + 1, :],
                                    start=True, stop=True)
                                bcs = bcpool.tile([DH, 512], f32, tag="bcs")
                                nc.vector.reciprocal_approx_fast(
                                    out=bcs[:], in_=bc[:])
                                o = opool.tile([DH, 512], f32, tag="o")
                                nc.vector.tensor_mul(o[:], u[0:DH, :],
                                                     bcs[:])
                                nc.sync.dma_start(
                                    out_d.ap()[h * DH:(h + 1) * DH,
                                               c * 512:(c + 1) * 512], o[:])

    nc.compile()
    return nc


def get_nc(nkv, nmk_attn, has_bv=True):
    key = (nkv, nmk_attn, has_bv)
    if key not in _CACHED:
        _CACHED[key] = _build_nc(nkv, nmk_attn, has_bv=has_bv)
    return _CACHED[key]


def make_in_maps(nkv, x, mask, wq, bq, wk, bk, wv, bv):
    x = np.ascontiguousarray(np.asarray(x, dtype=np.float32))
    mask = np.ascontiguousarray(np.asarray(mask, dtype=np.int32))
    wq = np.asarray(wq, dtype=np.float32)
    wk = np.asarray(wk, dtype=np.float32)
    wv = np.asarray(wv, dtype=np.float32)
    bq = np.asarray(bq, dtype=np.float32)
    bk = np.asarray(bk, dtype=np.float32)
    bv = np.asarray(bv, dtype=np.float32)
    # per-batch kv compaction indices (unmasked first, masked as padding)
    idx = []
    for b in range(B):
        on = np.flatnonzero(mask[b] != 0)
        off = np.flatnonzero(mask[b] == 0)
        ib = np.concatenate([on, off])[:nkv]
        idx.append(ib)
    in_maps = []
    for c in range(NCORES):
        b, g = c // 2, c % 2
        cols = slice(g * HD, (g + 1) * HD)
        xtb = np.ascontiguousarray(x[b].T)
        in_maps.append({
            "xt": xtb,
            "xkt": np.ascontiguousarray(xtb[:, idx[b]]),
            "wq": np.ascontiguousarray(wq[:, cols]),
            "wk": np.ascontiguousarray(wk[:, cols]),
            "wv": np.ascontiguousarray(wv[:, cols]),
            "bq": np.ascontiguousarray(bq[cols]),
            "bk": np.ascontiguousarray(bk[cols]),
            "bv": np.ascontiguousarray(bv[cols]),
            "maskc": np.ascontiguousarray(mask[b][idx[b]]),
        })
    return in_maps


def assemble_out(results):
    out = np.empty((B, S, H * DH), dtype=np.float32)
    for c in range(NCORES):
        b, g = c // 2, c % 2
        out[b, :, g * HD:(g + 1) * HD] = results[c]["out"].T
    return out


def pick_nkv(mask):
    mask = np.asarray(mask)
    nb_max = int((mask != 0).sum(axis=1).max())
    nmk_attn = max(1, -(-nb_max // 128))
    nkv = min(-(-(nmk_attn * 128) // 512) * 512, S)
    return nkv, nmk_attn


def run(trace=False, **inputs):
    from concourse.bass_utils import run_bass_kernel_spmd

    nkv, nmk_attn = pick_nkv(inputs["mask"])
    has_bv = bool(np.any(np.asarray(inputs["bv"])))
    nc = get_nc(nkv, nmk_attn, has_bv)
    in_maps = make_in_maps(nkv, **inputs)
    if not has_bv:
        for m in in_maps:
            m.pop("bv", None)
    res = run_bass_kernel_spmd(nc, in_maps, core_ids=list(range(NCORES)),
                               trace=trace)
    return assemble_out(res.results), res


def kernel(**inputs):
    out, _ = run(trace=False, **inputs)
    return out
